# revision 8
# baseline (speedup 1.0000x reference)
"""AttentionPairBias distributed Trainium2 kernel (8 NeuronCores), v2.

Sequence-parallel over query rows: core r owns i-rows [r*128, (r+1)*128).
z is host-cast to bf16 and reordered to [c, j, i_local] so each DMA is a
fully-contiguous 32KB/partition run and z arrives with c on partitions.

Pair-bias path (the expensive part) is computed with z as the MOVING
matmul operand against a small zero-padded stationary:
  stz[c, b, 10b+h] = wb2'[c,h] (h<8), 1/128 at col 10b+8   (mean)
  stq[c, b, 10b+9] = 1                                      (sum of z^2)
Eight band-matmuls (+ eight for z^2) accumulate into one PSUM bank
giving Bt[(b,h), (j,g)] for i = 8g+b -- B transposed, plus per-(i,j)
mean and sumsq rows.  Bands are PE-transposed in groups of 4 to get
[j, (g,b,h)] tiles.  Attention then runs in transposed [j, i] layout
(QK operands swapped); exp(logits) feeds attn@[V|1] directly as the
stationary, so no per-tile attention transposes and the softmax
denominator falls out of the ones column.

Math folding (host):
  z_ln @ Wb = rstd * (z @ Wb2) + v_h  with Wb2 = g_z*Wb - (g_z@Wb)/128,
  v_h = b_z@Wb constant per h cancels in softmax.  Q scale 1/sqrt(D)
  folded into Wq/bq.  Column mask folded into kt row 32 x qt ones row.
"""

import sys

if "/opt/trn_rl_repo" not in sys.path:
    sys.path.insert(0, "/opt/trn_rl_repo")

import numpy as np

L = 1024
CS = 256
CZ = 128
H = 8
D = 32
NCORES = 8
LR = L // NCORES  # 128
EPS = 1e-5
NB = 8            # i-bands per band-matmul group (i = 8g + b)
BH = 10           # meaningful cols per band: 8 heads + mean + sumsq
BP = 16           # padded cols per band (NB*BP = 128 -> FWL-eligible)
NGT = 16          # g values per j-chunk (128 i / NB)

_CACHE = {}
LAST_RESULT = None


def _build_graph():
    from contextlib import ExitStack

    import concourse.mybir as mybir
    import concourse.tile as tile
    from concourse import bacc
    from concourse.masks import make_identity

    f32 = mybir.dt.float32
    bf16 = mybir.dt.bfloat16
    AF = mybir.ActivationFunctionType

    nc = bacc.Bacc("TRN2", target_bir_lowering=False, debug=False)

    zc_e = nc.declare_dram_parameter("zc", [CZ, L, LR], bf16, isOutput=False)
    s_e = nc.declare_dram_parameter("s", [L, CS], f32, isOutput=False)
    sl_e = nc.declare_dram_parameter("sl", [LR, CS], f32, isOutput=False)
    wq_e = nc.declare_dram_parameter("wq", [CS, CS], bf16, isOutput=False)
    bq_e = nc.declare_dram_parameter("bq", [CS, 1], f32, isOutput=False)
    wk_e = nc.declare_dram_parameter("wk", [CS, CS], bf16, isOutput=False)
    wv_e = nc.declare_dram_parameter("wv", [CS, CS], bf16, isOutput=False)
    wg_e = nc.declare_dram_parameter("wg", [CS, CS], bf16, isOutput=False)
    wo_e = nc.declare_dram_parameter("wo", [CS, CS], bf16, isOutput=False)
    stz_e = nc.declare_dram_parameter("stz", [CZ, NB, NB * BP], bf16,
                                      isOutput=False)
    stq_e = nc.declare_dram_parameter("stq", [CZ, NB, NB * BP], bf16,
                                      isOutput=False)
    cm_e = nc.declare_dram_parameter("cm", [1, L], bf16, isOutput=False)
    rm_e = nc.declare_dram_parameter("rm", [LR, 1], f32, isOutput=False)
    gsb_e = nc.declare_dram_parameter("gsb", [128, CS], f32, isOutput=False)
    bsb_e = nc.declare_dram_parameter("bsb", [128, CS], f32, isOutput=False)
    out_e = nc.declare_dram_parameter("out", [LR, CS], f32, isOutput=True)

    with ExitStack() as ctx:
        tc = ctx.enter_context(tile.TileContext(nc))
        sing = ctx.enter_context(tc.tile_pool(name="sing", bufs=1))
        zp = ctx.enter_context(tc.tile_pool(name="zp", bufs=2))
        sqp = ctx.enter_context(tc.tile_pool(name="sqp", bufs=2))
        bsp = ctx.enter_context(tc.tile_pool(name="bsp", bufs=2))
        bbp = ctx.enter_context(tc.tile_pool(name="bbp", bufs=2))
        wkp = ctx.enter_context(tc.tile_pool(name="wkp", bufs=4))
        expp = ctx.enter_context(tc.tile_pool(name="expp", bufs=3))
        sp = ctx.enter_context(tc.tile_pool(name="sp", bufs=3))
        pB = ctx.enter_context(tc.tile_pool(name="pB", bufs=2, space="PSUM"))
        pT = ctx.enter_context(tc.tile_pool(name="pT", bufs=2, space="PSUM"))
        pP = ctx.enter_context(tc.tile_pool(name="pP", bufs=1, space="PSUM"))
        pO = ctx.enter_context(tc.tile_pool(name="pO", bufs=1, space="PSUM"))

        # ---------------- constants / params ----------------
        for cval in (0.0, 1.0, -1.0, -0.5, EPS):
            cl = sing.tile([128, 1], f32, tag=f"const_{cval}")
            nc.vector.memset(cl, cval)
            nc.const_aps.aps[(f32, cval)] = cl[:, :]

        ident = sing.tile([128, 128], f32)
        make_identity(nc, ident)

        stz_t = sing.tile([CZ, NB, NB * BP], bf16, tag="stz")
        nc.sync.dma_start(out=stz_t, in_=stz_e[:, :, :])
        stq_t = sing.tile([CZ, NB, NB * BP], bf16, tag="stq")
        nc.sync.dma_start(out=stq_t, in_=stq_e[:, :, :])
        ident_bf = sing.tile([128, 128], bf16)
        nc.scalar.activation(ident_bf, ident, AF.Copy)
        gsb_t = sing.tile([128, CS], f32)
        nc.sync.dma_start(out=gsb_t, in_=gsb_e[:, :])
        bsb_t = sing.tile([128, CS], f32)
        nc.sync.dma_start(out=bsb_t, in_=bsb_e[:, :])
        rm_t = sing.tile([LR, 1], f32)
        nc.sync.dma_start(out=rm_t, in_=rm_e[:, :])
        bq_t = sing.tile([128, 2], f32)
        nc.sync.dma_start(out=bq_t, in_=bq_e.rearrange("(k p) x -> p (k x)", p=128))

        wmats = {}
        for name, e, dt_ in (("wq", wq_e, bf16), ("wk", wk_e, bf16),
                             ("wv", wv_e, bf16), ("wg", wg_e, bf16),
                             ("wo", wo_e, bf16)):
            t = sing.tile([128, 2, CS], dt_, tag=f"w_{name}")
            nc.sync.dma_start(out=t, in_=e.rearrange("(k p) n -> p k n", p=128))
            wmats[name] = t

        # ---------------- persistent SBUF state ----------------
        slnT = sing.tile([128, 2, L], bf16)       # s_ln^T  [cs_in, j]
        slT = sing.tile([128, 2, LR], bf16)       # s_ln^T of local rows
        kt = sing.tile([33, H, L], bf16)          # [K_h^T ; cmask]
        qt = sing.tile([33, H, LR], bf16)         # [Q_h^T ; ones]
        vsb = sing.tile([128, NCORES, H, 33], bf16)  # per j-chunk [V_h | 1]
        gate_g = sing.tile([LR, CS], f32)         # sigmoid gate

        nc.vector.memset(vsb, 1.0)  # ones columns (:, :, :, 32) survive
        nc.vector.memset(qt[32:33, :, :], 1.0)

        # ---------------- s-side layernorm + transpose ----------------
        for t_i in range(9):
            src = s_e[t_i * 128:(t_i + 1) * 128, :] if t_i < 8 else sl_e[:, :]
            st_ = sp.tile([128, CS], f32, tag="st")
            nc.sync.dma_start(out=st_, in_=src)
            st6 = wkp.tile([128, 6], f32, tag="st6")
            nc.vector.bn_stats(out=st6, in_=st_)
            mv = wkp.tile([128, 2], f32, tag="mv")
            nc.vector.bn_aggr(out=mv, in_=st6)
            lnv = wkp.tile([128, 1], f32, tag="lnv")
            nc.scalar.activation(lnv, mv[:, 1:2], AF.Ln, bias=EPS)
            rstd1 = wkp.tile([128, 1], f32, tag="rstd1")
            nc.scalar.activation(rstd1, lnv, AF.Exp, scale=-0.5)
            negmr = wkp.tile([128, 1], f32, tag="negmr")
            nc.vector.tensor_mul(negmr, mv[:, 0:1], rstd1)
            nc.vector.tensor_scalar_mul(negmr, negmr, -1.0)
            slnt = sp.tile([128, CS], f32, tag="slnt")
            nc.scalar.activation(slnt, st_, AF.Identity, bias=negmr, scale=rstd1)
            nc.vector.tensor_mul(slnt, slnt, gsb_t)
            slnb = sp.tile([128, CS], bf16, tag="slnb")
            nc.vector.tensor_add(slnb, slnt, bsb_t)
            for k in range(2):
                pt = pT.tile([128, 4, NB, BP], bf16, tag="ptb", name="pt")
                nc.tensor.transpose(pt[:, 0, :, :],
                                    slnb[:, k * 128:(k + 1) * 128], ident_bf)
                if t_i < 8:
                    dst = slnT[:, k, t_i * 128:(t_i + 1) * 128]
                else:
                    dst = slT[:, k, :]
                nc.scalar.activation(dst, pt[:, 0, :, :], AF.Copy)

        # ---------------- K^T (+cmask row), bf16 ----------------
        for m in range(2):
            for jh in range(2):
                ps = pP.tile([128, 4, 128], f32, tag="lp", name="ps")
                for k in range(2):
                    nc.tensor.matmul(
                        ps[:, :, :], wmats["wk"][:, k, m * 128:(m + 1) * 128],
                        slnT[:, k, jh * 512:(jh + 1) * 512],
                        start=(k == 0), stop=(k == 1))
                for q in range(4):
                    h = m * 4 + q
                    nc.scalar.activation(
                        kt[0:32, h, jh * 512:(jh + 1) * 512],
                        ps[q * 32:(q + 1) * 32, :, :], AF.Copy)
        for h in range(H):
            nc.sync.dma_start(out=kt[32:33, h, :], in_=cm_e[0:1, :])

        # ---------------- Q^T (+ones row), folded scale/bias ----------
        for m in range(2):
            ps = pP.tile([128, 4, 128], f32, tag="lp", name="ps")
            for k in range(2):
                nc.tensor.matmul(
                    ps[:, 0, :], wmats["wq"][:, k, m * 128:(m + 1) * 128],
                    slT[:, k, :], start=(k == 0), stop=(k == 1))
            for q in range(4):
                h = m * 4 + q
                nc.scalar.activation(
                    qt[0:32, h, :], ps[q * 32:(q + 1) * 32, 0, :], AF.Identity,
                    bias=bq_t[q * 32:(q + 1) * 32, m:m + 1])

        # ---------------- V (interleaved [V_h | 1]) ----------------
        for jc in range(8):
            ps = pP.tile([128, 4, 128], f32, tag="lp", name="ps")
            for k in range(2):
                nc.tensor.matmul(
                    ps[:, 0:2, :], slnT[:, k, jc * 128:(jc + 1) * 128],
                    wmats["wv"][:, k, :], start=(k == 0), stop=(k == 1))
            for h in range(H):
                nc.scalar.activation(
                    vsb[:, jc, h, 0:32],
                    ps[:, h // 4, (h % 4) * 32:(h % 4) * 32 + 32], AF.Copy)

        # ---------------- G = sigmoid(s_ln @ Wg) ----------------
        psg = pP.tile([128, 4, 128], f32, tag="lp", name="psg")
        for k in range(2):
            nc.tensor.matmul(psg[:, 0:2, :], slT[:, k, :], wmats["wg"][:, k, :],
                             start=(k == 0), stop=(k == 1))
        eg = sp.tile([128, CS], f32, tag="eg")
        nc.scalar.activation(eg, psg[:, 0:2, :], AF.Exp, scale=-1.0)
        nc.vector.tensor_scalar_add(eg, eg, 1.0)
        nc.vector.reciprocal(gate_g, eg)

        # ---------------- main z loop ----------------
        o_ps = pO.tile([128, H, 33], f32)

        for jc in range(8):
            j0 = jc * 128
            zt = zp.tile([CZ, 128, NGT, NB], bf16, tag="zt")  # [c, j, g, b]
            nc.sync.dma_start(out=zt, in_=zc_e[:, j0:j0 + 128, :])

            b_sb = bbp.tile([128, NGT, NB, BH], bf16, tag="bsb")  # [j,g,b,h']

            for half in range(2):
                g_half = 8 * half
                zq = sqp.tile([CZ, 128, 8, NB], bf16, tag="zq")
                nc.vector.tensor_mul(
                    zq, zt[:, :, g_half:g_half + 8, :],
                    zt[:, :, g_half:g_half + 8, :])
                bkt = pB.tile([128, 2, 128, 4], f32, tag="bB", name="bkt")
                for b in range(NB):
                    for q in range(2):
                        g0 = 8 * half + 4 * q
                        nc.tensor.matmul(
                            bkt[:, q, :, :], stz_t[:, b, :],
                            zt[:, :, g0:g0 + 4, b],
                            start=(b == 0), stop=False, skip_group_check=True)
                for b in range(NB):
                    for q in range(2):
                        nc.tensor.matmul(
                            bkt[:, q, :, :], stq_t[:, b, :],
                            zq[:, :, 4 * q:4 * q + 4, b],
                            start=False, stop=(b == NB - 1),
                            skip_group_check=True)
                bs = bsp.tile([128, 128, 8], bf16, tag="bs")
                nc.scalar.activation(bs[:, :, 0:4], bkt[:, 0], AF.Copy)
                nc.scalar.activation(bs[:, :, 4:8], bkt[:, 1], AF.Copy)
                for tt in range(2):
                    ptb = pT.tile([128, 4, NB, BP], bf16, tag="ptb")
                    for u in range(4):
                        nc.tensor.transpose(
                            ptb[:, u, :, :], bs[:, :, 4 * tt + u], ident_bf)
                    g_abs = 8 * half + 4 * tt
                    nc.vector.tensor_copy(
                        out=b_sb[:, g_abs:g_abs + 4, :, :],
                        in_=ptb[:, :, :, 0:BH])

            # ---- rstd^T for this chunk: var = sumsq/128 - mean^2 ----
            mean = b_sb[:, :, :, 8]
            ssq = b_sb[:, :, :, 9]
            m2 = wkp.tile([128, NGT, NB], f32, tag="m2")
            nc.vector.tensor_mul(m2, mean, mean)
            vr = wkp.tile([128, NGT, NB], f32, tag="vr")
            nc.vector.tensor_scalar_mul(vr, ssq, 1.0 / CZ)
            nc.vector.tensor_sub(vr, vr, m2)
            lnv2 = wkp.tile([128, NGT, NB], f32, tag="lnv2")
            nc.scalar.activation(lnv2, vr, AF.Ln, bias=EPS)
            rstd = wkp.tile([128, NGT, NB], f32, tag="rstd")
            nc.scalar.activation(rstd, lnv2, AF.Exp, scale=-0.5)

            # ---- attention for this chunk, transposed [j, i] ----
            for hq in range(2):
                lp = pP.tile([128, 4, 128], f32, tag="lp")
                for hh in range(4):
                    h = 4 * hq + hh
                    nc.tensor.matmul(lp[:, hh, :], kt[:, h, j0:j0 + 128],
                                     qt[:, h, :], start=True, stop=True)
                for hh in range(4):
                    h = 4 * hq + hh
                    rb = wkp.tile([128, NGT, NB], f32, tag="rb")
                    nc.gpsimd.tensor_mul(rb, b_sb[:, :, :, h], rstd)
                    lg = wkp.tile([128, 128], f32, tag="lg")
                    nc.vector.tensor_add(lg, rb, lp[:, hh, :])
                    ex = expp.tile([128, 128], bf16, tag="ex")
                    nc.scalar.activation(ex, lg, AF.Exp)
                    nc.tensor.matmul(
                        o_ps[:, h, :], ex, vsb[:, jc, h, :],
                        start=(jc == 0 and h == 0),
                        stop=(jc == 7 and h == 7), skip_group_check=True)

        # ---------------- epilogue ----------------
        den = sing.tile([128, H], f32)
        nc.scalar.activation(den, o_ps[:, :, 32], AF.Copy)
        rec = sing.tile([128, H], f32)
        nc.vector.reciprocal(rec, den)
        gated = sing.tile([128, CS], f32)
        for h in range(H):
            nc.vector.tensor_scalar_mul(
                gated[:, h * 32:(h + 1) * 32],
                o_ps[:, h, 0:32], rec[:, h:h + 1])
        nc.vector.tensor_mul(gated, gated, gate_g)
        gated_b = sing.tile([128, CS], bf16)
        nc.scalar.activation(gated_b, gated, AF.Copy)
        gts = []
        for k in range(2):
            pt = pT.tile([128, 4, NB, BP], bf16, tag="ptb", name="pt")
            nc.tensor.transpose(pt[:, 0, :, :],
                                gated_b[:, k * 128:(k + 1) * 128], ident_bf)
            gt = wkp.tile([128, 128], bf16, tag=f"gt{k}")
            nc.scalar.activation(gt, pt[:, 0, :, :], AF.Copy)
            gts.append(gt)
        dsps = pP.tile([128, 4, 128], f32, tag="lp", name="ps")
        for k in range(2):
            nc.tensor.matmul(dsps[:, 0:2, :], gts[k], wmats["wo"][:, k, :],
                             start=(k == 0), stop=(k == 1))
        dsb = sing.tile([128, CS], f32)
        nc.scalar.activation(dsb, dsps[:, 0:2, :], AF.Copy)
        nc.vector.tensor_scalar_mul(dsb, dsb, rm_t[:, 0:1])
        nc.sync.dma_start(out=out_e[:, :], in_=dsb)

    nc.compile()
    return nc


def _host_prep(s, z, res_mask, g_s, b_s, g_z, b_z, Wq, bq, Wk, Wv, Wb, Wg, Wo):
    import ml_dtypes
    bf16 = ml_dtypes.bfloat16

    s = np.ascontiguousarray(np.asarray(s, np.float32))
    res_mask = np.asarray(res_mask).astype(bool)
    g_s = np.asarray(g_s, np.float32)
    b_s = np.asarray(b_s, np.float32)
    g_z = np.asarray(g_z, np.float32)
    b_z = np.asarray(b_z, np.float32)
    scale = 1.0 / np.sqrt(D)
    wqp = (np.asarray(Wq, np.float32) * scale).astype(bf16)
    bqp = np.ascontiguousarray(
        (np.asarray(bq, np.float32) * scale).reshape(CS, 1))
    wb = np.asarray(Wb, np.float32)
    w1 = g_z[:, None] * wb
    u = g_z @ wb
    wb2 = (w1 - u[None, :] / CZ).astype(np.float32)  # [CZ, H]

    # band stationaries [c, b, 128] (16 cols per band, 10 meaningful)
    stz = np.zeros((CZ, NB, NB * BP), np.float32)
    stq = np.zeros((CZ, NB, NB * BP), np.float32)
    for b in range(NB):
        stz[:, b, BP * b:BP * b + H] = wb2
        stz[:, b, BP * b + 8] = 1.0 / CZ
        stq[:, b, BP * b + 9] = 1.0

    cmask = np.where(res_mask, 0.0, -1e30).astype(np.float32).reshape(1, L)
    rmask = res_mask.astype(np.float32)
    gsb = np.ascontiguousarray(np.broadcast_to(g_s, (128, CS)))
    bsb = np.ascontiguousarray(np.broadcast_to(b_s, (128, CS)))

    common = {
        "s": s,
        "wq": np.ascontiguousarray(wqp),
        "bq": bqp,
        "wk": np.ascontiguousarray(np.asarray(Wk, np.float32).astype(bf16)),
        "wv": np.ascontiguousarray(np.asarray(Wv, np.float32).astype(bf16)),
        "wg": np.ascontiguousarray(np.asarray(Wg, np.float32).astype(bf16)),
        "wo": np.ascontiguousarray(np.asarray(Wo, np.float32).astype(bf16)),
        "stz": np.ascontiguousarray(stz.astype(bf16)),
        "stq": np.ascontiguousarray(stq.astype(bf16)),
        "cm": np.ascontiguousarray(cmask.astype(bf16)),
        "gsb": gsb, "bsb": bsb,
    }

    zb = np.asarray(z, np.float32).astype(bf16)  # [i, j, c] bf16
    in_maps = []
    for r in range(NCORES):
        zc_r = np.ascontiguousarray(
            zb[r * LR:(r + 1) * LR].transpose(2, 1, 0))  # [c, j, i_local]
        m = dict(common)
        m["zc"] = zc_r
        m["sl"] = np.ascontiguousarray(s[r * LR:(r + 1) * LR])
        m["rm"] = np.ascontiguousarray(
            rmask[r * LR:(r + 1) * LR].reshape(LR, 1))
        in_maps.append(m)
    return in_maps


def kernel(s, z, res_mask, g_s, b_s, g_z, b_z, Wq, bq, Wk, Wv, Wb, Wg, Wo):
    global LAST_RESULT
    in_maps = _host_prep(s, z, res_mask, g_s, b_s, g_z, b_z,
                         Wq, bq, Wk, Wv, Wb, Wg, Wo)

    if "nc" not in _CACHE:
        _CACHE["nc"] = _build_graph()
    nc = _CACHE["nc"]

    from concourse.bass_utils import run_bass_kernel_spmd

    import os
    tmpdir = os.environ.get("BASS_TMPDIR")
    if tmpdir:
        os.makedirs(tmpdir, exist_ok=True)
    res = run_bass_kernel_spmd(nc, in_maps, core_ids=list(range(NCORES)),
                               tmpdir=tmpdir)
    LAST_RESULT = res
    out = np.concatenate([np.asarray(res.results[r]["out"])
                          for r in range(NCORES)], axis=0)
    return out.astype(np.float32)


# revision 9
# speedup vs baseline: 2.4232x; 2.4232x over previous
"""AttentionPairBias distributed Trainium2 kernel (8 NeuronCores), v2.

Sequence-parallel over query rows: core r owns i-rows [r*128, (r+1)*128).
z is host-cast to bf16 and reordered to [c, j, i_local] so each DMA is a
fully-contiguous 32KB/partition run and z arrives with c on partitions.

Pair-bias path (the expensive part) is computed with z as the MOVING
matmul operand against a small zero-padded stationary:
  stz[c, b, 10b+h] = wb2'[c,h] (h<8), 1/128 at col 10b+8   (mean)
  stq[c, b, 10b+9] = 1                                      (sum of z^2)
Eight band-matmuls (+ eight for z^2) accumulate into one PSUM bank
giving Bt[(b,h), (j,g)] for i = 8g+b -- B transposed, plus per-(i,j)
mean and sumsq rows.  Bands are PE-transposed in groups of 4 to get
[j, (g,b,h)] tiles.  Attention then runs in transposed [j, i] layout
(QK operands swapped); exp(logits) feeds attn@[V|1] directly as the
stationary, so no per-tile attention transposes and the softmax
denominator falls out of the ones column.

Math folding (host):
  z_ln @ Wb = rstd * (z @ Wb2) + v_h  with Wb2 = g_z*Wb - (g_z@Wb)/128,
  v_h = b_z@Wb constant per h cancels in softmax.  Q scale 1/sqrt(D)
  folded into Wq/bq.  Column mask folded into kt row 32 x qt ones row.
"""

import sys

if "/opt/trn_rl_repo" not in sys.path:
    sys.path.insert(0, "/opt/trn_rl_repo")

import numpy as np

L = 1024
CS = 256
CZ = 128
H = 8
D = 32
NCORES = 8
LR = L // NCORES  # 128
EPS = 1e-5
NB = 8            # i-bands per band-matmul group (i = 8g + b)
BH = 10           # meaningful cols per band: 8 heads + mean + sumsq
BP = 16           # padded cols per band (NB*BP = 128 -> FWL-eligible)
NGT = 16          # g values per j-chunk (128 i / NB)

_CACHE = {}
LAST_RESULT = None


def _build_graph():
    from contextlib import ExitStack

    import concourse.mybir as mybir
    import concourse.tile as tile
    from concourse import bacc
    from concourse.masks import make_identity

    f32 = mybir.dt.float32
    bf16 = mybir.dt.bfloat16
    AF = mybir.ActivationFunctionType

    nc = bacc.Bacc("TRN2", target_bir_lowering=False, debug=False)

    zc_e = nc.declare_dram_parameter("zc", [8, CZ, LR, 128], bf16,
                                     isOutput=False)
    s_e = nc.declare_dram_parameter("s", [L, CS], f32, isOutput=False)
    sl_e = nc.declare_dram_parameter("sl", [LR, CS], f32, isOutput=False)
    wq_e = nc.declare_dram_parameter("wq", [CS, CS], bf16, isOutput=False)
    bq_e = nc.declare_dram_parameter("bq", [CS, 1], f32, isOutput=False)
    wk_e = nc.declare_dram_parameter("wk", [CS, CS], bf16, isOutput=False)
    wv_e = nc.declare_dram_parameter("wv", [CS, CS], bf16, isOutput=False)
    wg_e = nc.declare_dram_parameter("wg", [CS, CS], bf16, isOutput=False)
    wo_e = nc.declare_dram_parameter("wo", [CS, CS], bf16, isOutput=False)
    stz_e = nc.declare_dram_parameter("stz", [CZ, NB, NB * BP], bf16,
                                      isOutput=False)
    stq_e = nc.declare_dram_parameter("stq", [CZ, NB, NB * BP], bf16,
                                      isOutput=False)
    cm_e = nc.declare_dram_parameter("cm", [1, L], bf16, isOutput=False)
    rm_e = nc.declare_dram_parameter("rm", [LR, 1], f32, isOutput=False)
    gsb_e = nc.declare_dram_parameter("gsb", [128, CS], f32, isOutput=False)
    bsb_e = nc.declare_dram_parameter("bsb", [128, CS], f32, isOutput=False)
    out_e = nc.declare_dram_parameter("out", [LR, CS], f32, isOutput=True)

    with ExitStack() as ctx:
        tc = ctx.enter_context(tile.TileContext(nc))
        sing = ctx.enter_context(tc.tile_pool(name="sing", bufs=1))
        zp = ctx.enter_context(tc.tile_pool(name="zp", bufs=2))
        sqp = ctx.enter_context(tc.tile_pool(name="sqp", bufs=2))
        bsp = ctx.enter_context(tc.tile_pool(name="bsp", bufs=2))
        bbp = ctx.enter_context(tc.tile_pool(name="bbp", bufs=2))
        wkp = ctx.enter_context(tc.tile_pool(name="wkp", bufs=4))
        expp = ctx.enter_context(tc.tile_pool(name="expp", bufs=3))
        sp = ctx.enter_context(tc.tile_pool(name="sp", bufs=3))
        pB = ctx.enter_context(tc.tile_pool(name="pB", bufs=2, space="PSUM"))
        pT = ctx.enter_context(tc.tile_pool(name="pT", bufs=2, space="PSUM"))
        pP = ctx.enter_context(tc.tile_pool(name="pP", bufs=1, space="PSUM"))
        pO = ctx.enter_context(tc.tile_pool(name="pO", bufs=1, space="PSUM"))

        # ---------------- constants / params ----------------
        for cval in (0.0, 1.0, -1.0, -0.5, EPS):
            cl = sing.tile([128, 1], f32, tag=f"const_{cval}")
            nc.vector.memset(cl, cval)
            nc.const_aps.aps[(f32, cval)] = cl[:, :]

        ident = sing.tile([128, 128], f32)
        make_identity(nc, ident)

        stz_t = sing.tile([CZ, NB, NB * BP], bf16, tag="stz")
        nc.sync.dma_start(out=stz_t, in_=stz_e[:, :, :])
        stq_t = sing.tile([CZ, NB, NB * BP], bf16, tag="stq")
        nc.sync.dma_start(out=stq_t, in_=stq_e[:, :, :])
        ident_bf = sing.tile([128, 128], bf16)
        nc.scalar.activation(ident_bf, ident, AF.Copy)
        gsb_t = sing.tile([128, CS], f32)
        nc.sync.dma_start(out=gsb_t, in_=gsb_e[:, :])
        bsb_t = sing.tile([128, CS], f32)
        nc.sync.dma_start(out=bsb_t, in_=bsb_e[:, :])
        rm_t = sing.tile([LR, 1], f32)
        nc.sync.dma_start(out=rm_t, in_=rm_e[:, :])
        bq_t = sing.tile([128, 2], f32)
        nc.sync.dma_start(out=bq_t, in_=bq_e.rearrange("(k p) x -> p (k x)", p=128))

        wmats = {}
        for name, e, dt_ in (("wq", wq_e, bf16), ("wk", wk_e, bf16),
                             ("wv", wv_e, bf16), ("wg", wg_e, bf16),
                             ("wo", wo_e, bf16)):
            t = sing.tile([128, 2, CS], dt_, tag=f"w_{name}")
            nc.sync.dma_start(out=t, in_=e.rearrange("(k p) n -> p k n", p=128))
            wmats[name] = t

        # ---------------- persistent SBUF state ----------------
        slnT = sing.tile([128, 2, L], bf16)       # s_ln^T  [cs_in, j]
        slT = sing.tile([128, 2, LR], bf16)       # s_ln^T of local rows
        kt = sing.tile([33, H, L], bf16)          # [K_h^T ; cmask]
        qt = sing.tile([33, H, LR], bf16)         # [Q_h^T ; ones]
        vsb = sing.tile([128, NCORES, H, 33], bf16)  # per j-chunk [V_h | 1]
        gate_g = sing.tile([LR, CS], f32)         # sigmoid gate

        nc.vector.memset(vsb, 1.0)  # ones columns (:, :, :, 32) survive
        nc.vector.memset(qt[32:33, :, :], 1.0)

        # ---------------- s-side layernorm + transpose ----------------
        for t_i in range(9):
            src = s_e[t_i * 128:(t_i + 1) * 128, :] if t_i < 8 else sl_e[:, :]
            st_ = sp.tile([128, CS], f32, tag="st")
            nc.sync.dma_start(out=st_, in_=src)
            st6 = wkp.tile([128, 6], f32, tag="st6")
            nc.vector.bn_stats(out=st6, in_=st_)
            mv = wkp.tile([128, 2], f32, tag="mv")
            nc.vector.bn_aggr(out=mv, in_=st6)
            lnv = wkp.tile([128, 1], f32, tag="lnv")
            nc.scalar.activation(lnv, mv[:, 1:2], AF.Ln, bias=EPS)
            rstd1 = wkp.tile([128, 1], f32, tag="rstd1")
            nc.scalar.activation(rstd1, lnv, AF.Exp, scale=-0.5)
            negmr = wkp.tile([128, 1], f32, tag="negmr")
            nc.vector.tensor_mul(negmr, mv[:, 0:1], rstd1)
            nc.vector.tensor_scalar_mul(negmr, negmr, -1.0)
            slnt = sp.tile([128, CS], f32, tag="slnt")
            nc.scalar.activation(slnt, st_, AF.Identity, bias=negmr, scale=rstd1)
            nc.vector.tensor_mul(slnt, slnt, gsb_t)
            slnb = sp.tile([128, CS], bf16, tag="slnb")
            nc.vector.tensor_add(slnb, slnt, bsb_t)
            for k in range(2):
                pt = pT.tile([128, 4, NB, BP], bf16, tag="ptb", name="pt")
                nc.tensor.transpose(pt[:, 0, :, :],
                                    slnb[:, k * 128:(k + 1) * 128], ident_bf)
                if t_i < 8:
                    dst = slnT[:, k, t_i * 128:(t_i + 1) * 128]
                else:
                    dst = slT[:, k, :]
                nc.scalar.activation(dst, pt[:, 0, :, :], AF.Copy)

        # ---------------- K^T (+cmask row), bf16 ----------------
        for m in range(2):
            for jh in range(2):
                ps = pP.tile([128, 4, 128], f32, tag="lp", name="ps")
                for k in range(2):
                    nc.tensor.matmul(
                        ps[:, :, :], wmats["wk"][:, k, m * 128:(m + 1) * 128],
                        slnT[:, k, jh * 512:(jh + 1) * 512],
                        start=(k == 0), stop=(k == 1))
                for q in range(4):
                    h = m * 4 + q
                    nc.scalar.activation(
                        kt[0:32, h, jh * 512:(jh + 1) * 512],
                        ps[q * 32:(q + 1) * 32, :, :], AF.Copy)
        for h in range(H):
            nc.sync.dma_start(out=kt[32:33, h, :], in_=cm_e[0:1, :])

        # ---------------- Q^T (+ones row), folded scale/bias ----------
        for m in range(2):
            ps = pP.tile([128, 4, 128], f32, tag="lp", name="ps")
            for k in range(2):
                nc.tensor.matmul(
                    ps[:, 0, :], wmats["wq"][:, k, m * 128:(m + 1) * 128],
                    slT[:, k, :], start=(k == 0), stop=(k == 1))
            for q in range(4):
                h = m * 4 + q
                nc.scalar.activation(
                    qt[0:32, h, :], ps[q * 32:(q + 1) * 32, 0, :], AF.Identity,
                    bias=bq_t[q * 32:(q + 1) * 32, m:m + 1])

        # ---------------- V (interleaved [V_h | 1]) ----------------
        for jc in range(8):
            ps = pP.tile([128, 4, 128], f32, tag="lp", name="ps")
            for k in range(2):
                nc.tensor.matmul(
                    ps[:, 0:2, :], slnT[:, k, jc * 128:(jc + 1) * 128],
                    wmats["wv"][:, k, :], start=(k == 0), stop=(k == 1))
            for h in range(H):
                nc.scalar.activation(
                    vsb[:, jc, h, 0:32],
                    ps[:, h // 4, (h % 4) * 32:(h % 4) * 32 + 32], AF.Copy)

        # ---------------- G = sigmoid(s_ln @ Wg) ----------------
        psg = pP.tile([128, 4, 128], f32, tag="lp", name="psg")
        for k in range(2):
            nc.tensor.matmul(psg[:, 0:2, :], slT[:, k, :], wmats["wg"][:, k, :],
                             start=(k == 0), stop=(k == 1))
        eg = sp.tile([128, CS], f32, tag="eg")
        nc.scalar.activation(eg, psg[:, 0:2, :], AF.Exp, scale=-1.0)
        nc.vector.tensor_scalar_add(eg, eg, 1.0)
        nc.vector.reciprocal(gate_g, eg)

        # ---------------- main z loop ----------------
        o_ps = pO.tile([128, H, 33], f32)

        for jc in range(8):
            j0 = jc * 128
            zt = zp.tile([CZ, LR, 128], bf16, tag="zt")  # [c, i, j]
            nc.sync.dma_start(out=zt, in_=zc_e[jc])

            b_sb = bbp.tile([128, 4, 4, NB, BH], bf16, tag="bsb")
            # [j, Q, ii, b, cc]; i = 32*Q + 4*b + ii
            b_v = b_sb.rearrange("j q i2 b c -> j q b i2 c")

            for half in range(2):
                i_h = 64 * half
                zq = sqp.tile([CZ, 64, 128], bf16, tag="zq")
                nc.vector.tensor_mul(
                    zq, zt[:, i_h:i_h + 64, :], zt[:, i_h:i_h + 64, :])
                bkt = pB.tile([128, 2, 4, 128], f32, tag="bB", name="bkt")
                for b in range(NB):
                    for q in range(2):
                        i0 = i_h + 32 * q + 4 * b
                        nc.tensor.matmul(
                            bkt[:, q, :, :], stz_t[:, b, :],
                            zt[:, i0:i0 + 4, :],
                            start=(b == 0), stop=False, skip_group_check=True)
                for b in range(NB):
                    for q in range(2):
                        i0 = 32 * q + 4 * b
                        nc.tensor.matmul(
                            bkt[:, q, :, :], stq_t[:, b, :],
                            zq[:, i0:i0 + 4, :],
                            start=False, stop=(b == NB - 1),
                            skip_group_check=True)
                bs = bsp.tile([128, 2, 4, 128], bf16, tag="bs")
                nc.scalar.activation(bs[:, 0], bkt[:, 0], AF.Copy)
                nc.scalar.activation(bs[:, 1], bkt[:, 1], AF.Copy)
                for q in range(2):
                    ptb = pT.tile([128, 4, NB, BP], bf16, tag="ptb")
                    for u in range(4):
                        nc.tensor.transpose(
                            ptb[:, u, :, :], bs[:, q, u, :], ident_bf)
                    nc.vector.tensor_copy(
                        out=b_sb[:, 2 * half + q, :, :, :],
                        in_=ptb[:, :, :, 0:BH])

            # ---- rstd^T for this chunk: var = sumsq/128 - mean^2 ----
            mean = b_v[:, :, :, :, 8]
            ssq = b_v[:, :, :, :, 9]
            m2 = wkp.tile([128, 128], f32, tag="m2")
            nc.vector.tensor_mul(m2, mean, mean)
            vr = wkp.tile([128, 128], f32, tag="vr")
            nc.vector.tensor_scalar_mul(vr, ssq, 1.0 / CZ)
            nc.vector.tensor_sub(vr, vr, m2)
            lnv2 = wkp.tile([128, 128], f32, tag="lnv2")
            nc.scalar.activation(lnv2, vr, AF.Ln, bias=EPS)
            rstd = wkp.tile([128, 128], f32, tag="rstd")
            nc.scalar.activation(rstd, lnv2, AF.Exp, scale=-0.5)

            # ---- attention for this chunk, transposed [j, i] ----
            for hq in range(2):
                lp = pP.tile([128, 4, 128], f32, tag="lp")
                for hh in range(4):
                    h = 4 * hq + hh
                    nc.tensor.matmul(lp[:, hh, :], kt[:, h, j0:j0 + 128],
                                     qt[:, h, :], start=True, stop=True)
                for hh in range(4):
                    h = 4 * hq + hh
                    rb = wkp.tile([128, 128], f32, tag="rb")
                    nc.gpsimd.tensor_mul(rb, b_v[:, :, :, :, h], rstd)
                    lg = wkp.tile([128, 128], f32, tag="lg")
                    nc.vector.tensor_add(lg, rb, lp[:, hh, :])
                    ex = expp.tile([128, 128], bf16, tag="ex")
                    nc.scalar.activation(ex, lg, AF.Exp)
                    nc.tensor.matmul(
                        o_ps[:, h, :], ex, vsb[:, jc, h, :],
                        start=(jc == 0 and h == 0),
                        stop=(jc == 7 and h == 7), skip_group_check=True)

        # ---------------- epilogue ----------------
        den = sing.tile([128, H], f32)
        nc.scalar.activation(den, o_ps[:, :, 32], AF.Copy)
        rec = sing.tile([128, H], f32)
        nc.vector.reciprocal(rec, den)
        gated = sing.tile([128, CS], f32)
        for h in range(H):
            nc.vector.tensor_scalar_mul(
                gated[:, h * 32:(h + 1) * 32],
                o_ps[:, h, 0:32], rec[:, h:h + 1])
        nc.vector.tensor_mul(gated, gated, gate_g)
        gated_b = sing.tile([128, CS], bf16)
        nc.scalar.activation(gated_b, gated, AF.Copy)
        gts = []
        for k in range(2):
            pt = pT.tile([128, 4, NB, BP], bf16, tag="ptb", name="pt")
            nc.tensor.transpose(pt[:, 0, :, :],
                                gated_b[:, k * 128:(k + 1) * 128], ident_bf)
            gt = wkp.tile([128, 128], bf16, tag=f"gt{k}")
            nc.scalar.activation(gt, pt[:, 0, :, :], AF.Copy)
            gts.append(gt)
        dsps = pP.tile([128, 4, 128], f32, tag="lp", name="ps")
        for k in range(2):
            nc.tensor.matmul(dsps[:, 0:2, :], gts[k], wmats["wo"][:, k, :],
                             start=(k == 0), stop=(k == 1))
        dsb = sing.tile([128, CS], f32)
        nc.scalar.activation(dsb, dsps[:, 0:2, :], AF.Copy)
        nc.vector.tensor_scalar_mul(dsb, dsb, rm_t[:, 0:1])
        nc.sync.dma_start(out=out_e[:, :], in_=dsb)

    nc.compile()
    return nc


def _host_prep(s, z, res_mask, g_s, b_s, g_z, b_z, Wq, bq, Wk, Wv, Wb, Wg, Wo):
    import ml_dtypes
    bf16 = ml_dtypes.bfloat16

    s = np.ascontiguousarray(np.asarray(s, np.float32))
    res_mask = np.asarray(res_mask).astype(bool)
    g_s = np.asarray(g_s, np.float32)
    b_s = np.asarray(b_s, np.float32)
    g_z = np.asarray(g_z, np.float32)
    b_z = np.asarray(b_z, np.float32)
    scale = 1.0 / np.sqrt(D)
    wqp = (np.asarray(Wq, np.float32) * scale).astype(bf16)
    bqp = np.ascontiguousarray(
        (np.asarray(bq, np.float32) * scale).reshape(CS, 1))
    wb = np.asarray(Wb, np.float32)
    w1 = g_z[:, None] * wb
    u = g_z @ wb
    wb2 = (w1 - u[None, :] / CZ).astype(np.float32)  # [CZ, H]

    # band stationaries [c, b, 128] (16 cols per band, 10 meaningful)
    stz = np.zeros((CZ, NB, NB * BP), np.float32)
    stq = np.zeros((CZ, NB, NB * BP), np.float32)
    for b in range(NB):
        stz[:, b, BP * b:BP * b + H] = wb2
        stz[:, b, BP * b + 8] = 1.0 / CZ
        stq[:, b, BP * b + 9] = 1.0

    cmask = np.where(res_mask, 0.0, -1e30).astype(np.float32).reshape(1, L)
    rmask = res_mask.astype(np.float32)
    gsb = np.ascontiguousarray(np.broadcast_to(g_s, (128, CS)))
    bsb = np.ascontiguousarray(np.broadcast_to(b_s, (128, CS)))

    common = {
        "s": s,
        "wq": np.ascontiguousarray(wqp),
        "bq": bqp,
        "wk": np.ascontiguousarray(np.asarray(Wk, np.float32).astype(bf16)),
        "wv": np.ascontiguousarray(np.asarray(Wv, np.float32).astype(bf16)),
        "wg": np.ascontiguousarray(np.asarray(Wg, np.float32).astype(bf16)),
        "wo": np.ascontiguousarray(np.asarray(Wo, np.float32).astype(bf16)),
        "stz": np.ascontiguousarray(stz.astype(bf16)),
        "stq": np.ascontiguousarray(stq.astype(bf16)),
        "cm": np.ascontiguousarray(cmask.astype(bf16)),
        "gsb": gsb, "bsb": bsb,
    }

    zb = np.asarray(z, np.float32).astype(bf16)  # [i, j, c] bf16
    in_maps = []
    for r in range(NCORES):
        zc_r = np.ascontiguousarray(
            zb[r * LR:(r + 1) * LR].reshape(LR, 8, 128, CZ)
            .transpose(1, 3, 0, 2))  # [jc, c, i_local, j]
        m = dict(common)
        m["zc"] = zc_r
        m["sl"] = np.ascontiguousarray(s[r * LR:(r + 1) * LR])
        m["rm"] = np.ascontiguousarray(
            rmask[r * LR:(r + 1) * LR].reshape(LR, 1))
        in_maps.append(m)
    return in_maps


def kernel(s, z, res_mask, g_s, b_s, g_z, b_z, Wq, bq, Wk, Wv, Wb, Wg, Wo):
    global LAST_RESULT
    in_maps = _host_prep(s, z, res_mask, g_s, b_s, g_z, b_z,
                         Wq, bq, Wk, Wv, Wb, Wg, Wo)

    if "nc" not in _CACHE:
        _CACHE["nc"] = _build_graph()
    nc = _CACHE["nc"]

    from concourse.bass_utils import run_bass_kernel_spmd

    import os
    tmpdir = os.environ.get("BASS_TMPDIR")
    if tmpdir:
        os.makedirs(tmpdir, exist_ok=True)
    res = run_bass_kernel_spmd(nc, in_maps, core_ids=list(range(NCORES)),
                               tmpdir=tmpdir)
    LAST_RESULT = res
    out = np.concatenate([np.asarray(res.results[r]["out"])
                          for r in range(NCORES)], axis=0)
    return out.astype(np.float32)


# revision 11
# speedup vs baseline: 2.8127x; 1.1607x over previous
"""AttentionPairBias distributed Trainium2 kernel (8 NeuronCores), v2.

Sequence-parallel over query rows: core r owns i-rows [r*128, (r+1)*128).
z is host-cast to bf16 and reordered to [c, j, i_local] so each DMA is a
fully-contiguous 32KB/partition run and z arrives with c on partitions.

Pair-bias path (the expensive part) is computed with z as the MOVING
matmul operand against a small zero-padded stationary:
  stz[c, b, 10b+h] = wb2'[c,h] (h<8), 1/128 at col 10b+8   (mean)
  stq[c, b, 10b+9] = 1                                      (sum of z^2)
Eight band-matmuls (+ eight for z^2) accumulate into one PSUM bank
giving Bt[(b,h), (j,g)] for i = 8g+b -- B transposed, plus per-(i,j)
mean and sumsq rows.  Bands are PE-transposed in groups of 4 to get
[j, (g,b,h)] tiles.  Attention then runs in transposed [j, i] layout
(QK operands swapped); exp(logits) feeds attn@[V|1] directly as the
stationary, so no per-tile attention transposes and the softmax
denominator falls out of the ones column.

Math folding (host):
  z_ln @ Wb = rstd * (z @ Wb2) + v_h  with Wb2 = g_z*Wb - (g_z@Wb)/128,
  v_h = b_z@Wb constant per h cancels in softmax.  Q scale 1/sqrt(D)
  folded into Wq/bq.  Column mask folded into kt row 32 x qt ones row.
"""

import sys

if "/opt/trn_rl_repo" not in sys.path:
    sys.path.insert(0, "/opt/trn_rl_repo")

import numpy as np

L = 1024
CS = 256
CZ = 128
H = 8
D = 32
NCORES = 8
LR = L // NCORES  # 128
EPS = 1e-5
NB = 8            # i-bands per band-matmul group (i = 8g + b)
BH = 10           # meaningful cols per band: 8 heads + mean + sumsq
BP = 16           # padded cols per band (NB*BP = 128 -> FWL-eligible)
NGT = 16          # g values per j-chunk (128 i / NB)

_CACHE = {}
LAST_RESULT = None


def _build_graph():
    from contextlib import ExitStack

    import concourse.mybir as mybir
    import concourse.tile as tile
    from concourse import bacc
    from concourse.masks import make_identity

    f32 = mybir.dt.float32
    bf16 = mybir.dt.bfloat16
    AF = mybir.ActivationFunctionType

    nc = bacc.Bacc("TRN2", target_bir_lowering=False, debug=False)

    zc_e = nc.declare_dram_parameter("zc", [8, CZ, LR, 128], bf16,
                                     isOutput=False)
    s_e = nc.declare_dram_parameter("s", [L, CS], f32, isOutput=False)
    sl_e = nc.declare_dram_parameter("sl", [LR, CS], f32, isOutput=False)
    wq_e = nc.declare_dram_parameter("wq", [CS, CS], bf16, isOutput=False)
    bq_e = nc.declare_dram_parameter("bq", [CS, 1], f32, isOutput=False)
    wk_e = nc.declare_dram_parameter("wk", [CS, CS], bf16, isOutput=False)
    wv_e = nc.declare_dram_parameter("wv", [CS, CS], bf16, isOutput=False)
    wg_e = nc.declare_dram_parameter("wg", [CS, CS], bf16, isOutput=False)
    wo_e = nc.declare_dram_parameter("wo", [CS, CS], bf16, isOutput=False)
    stz_e = nc.declare_dram_parameter("stz", [CZ, NB, NB * BP], bf16,
                                      isOutput=False)
    stq_e = nc.declare_dram_parameter("stq", [CZ, NB, NB * BP], bf16,
                                      isOutput=False)
    cm_e = nc.declare_dram_parameter("cm", [1, L], bf16, isOutput=False)
    rm_e = nc.declare_dram_parameter("rm", [LR, 1], f32, isOutput=False)
    gsb_e = nc.declare_dram_parameter("gsb", [128, CS], f32, isOutput=False)
    bsb_e = nc.declare_dram_parameter("bsb", [128, CS], f32, isOutput=False)
    out_e = nc.declare_dram_parameter("out", [LR, CS], f32, isOutput=True)

    with ExitStack() as ctx:
        tc = ctx.enter_context(tile.TileContext(nc))
        sing = ctx.enter_context(tc.tile_pool(name="sing", bufs=1))
        zp = ctx.enter_context(tc.tile_pool(name="zp", bufs=2))
        sqp = ctx.enter_context(tc.tile_pool(name="sqp", bufs=2))
        bsp = ctx.enter_context(tc.tile_pool(name="bsp", bufs=2))
        bbp = ctx.enter_context(tc.tile_pool(name="bbp", bufs=2))
        wkp = ctx.enter_context(tc.tile_pool(name="wkp", bufs=4))
        expp = ctx.enter_context(tc.tile_pool(name="expp", bufs=3))
        sp = ctx.enter_context(tc.tile_pool(name="sp", bufs=3))
        pB = ctx.enter_context(tc.tile_pool(name="pB", bufs=2, space="PSUM"))
        pT = ctx.enter_context(tc.tile_pool(name="pT", bufs=2, space="PSUM"))
        pP = ctx.enter_context(tc.tile_pool(name="pP", bufs=1, space="PSUM"))
        pO = ctx.enter_context(tc.tile_pool(name="pO", bufs=1, space="PSUM"))

        # ---------------- constants / params ----------------
        for cval in (0.0, 1.0, -1.0, -0.5, EPS):
            cl = sing.tile([128, 1], f32, tag=f"const_{cval}")
            nc.vector.memset(cl, cval)
            nc.const_aps.aps[(f32, cval)] = cl[:, :]

        ident = sing.tile([128, 128], f32)
        make_identity(nc, ident)

        stz_t = sing.tile([CZ, NB, NB * BP], bf16, tag="stz")
        nc.sync.dma_start(out=stz_t, in_=stz_e[:, :, :])
        stq_t = sing.tile([CZ, NB, NB * BP], bf16, tag="stq")
        nc.sync.dma_start(out=stq_t, in_=stq_e[:, :, :])
        ident_bf = sing.tile([128, 128], bf16)
        nc.scalar.activation(ident_bf, ident, AF.Copy)
        gsb_t = sing.tile([128, CS], f32)
        nc.sync.dma_start(out=gsb_t, in_=gsb_e[:, :])
        bsb_t = sing.tile([128, CS], f32)
        nc.sync.dma_start(out=bsb_t, in_=bsb_e[:, :])
        rm_t = sing.tile([LR, 1], f32)
        nc.sync.dma_start(out=rm_t, in_=rm_e[:, :])
        bq_t = sing.tile([128, 2], f32)
        nc.sync.dma_start(out=bq_t, in_=bq_e.rearrange("(k p) x -> p (k x)", p=128))

        wmats = {}
        for name, e, dt_ in (("wq", wq_e, bf16), ("wk", wk_e, bf16),
                             ("wv", wv_e, bf16), ("wg", wg_e, bf16),
                             ("wo", wo_e, bf16)):
            t = sing.tile([128, 2, CS], dt_, tag=f"w_{name}")
            nc.sync.dma_start(out=t, in_=e.rearrange("(k p) n -> p k n", p=128))
            wmats[name] = t

        # ---------------- persistent SBUF state ----------------
        slnT = sing.tile([128, 2, L], bf16)       # s_ln^T  [cs_in, j]
        slT = sing.tile([128, 2, LR], bf16)       # s_ln^T of local rows
        kt = sing.tile([33, H, L], bf16)          # [K_h^T ; cmask]
        qt = sing.tile([33, H, LR], bf16)         # [Q_h^T ; ones]
        vsb = sing.tile([128, NCORES, H, 33], bf16)  # per j-chunk [V_h | 1]
        gate_g = sing.tile([LR, CS], f32)         # sigmoid gate

        nc.vector.memset(vsb, 1.0)  # ones columns (:, :, :, 32) survive
        nc.vector.memset(qt[32:33, :, :], 1.0)

        # ---------------- s-side layernorm + transpose ----------------
        sts = []
        mvall = sing.tile([128, 9, 2], f32)
        for t_i in range(9):
            src = s_e[t_i * 128:(t_i + 1) * 128, :] if t_i < 8 else sl_e[:, :]
            st_ = sp.tile([128, CS], f32, tag=f"st{t_i}")
            nc.sync.dma_start(out=st_, in_=src)
            sts.append(st_)
            st6 = wkp.tile([128, 6], f32, tag="st6")
            nc.vector.bn_stats(out=st6, in_=st_)
            nc.vector.bn_aggr(out=mvall[:, t_i, :], in_=st6)
        lnv9 = sing.tile([128, 9], f32)
        nc.scalar.activation(lnv9, mvall[:, :, 1], AF.Ln, bias=EPS)
        rstd9 = sing.tile([128, 9], f32)
        nc.scalar.activation(rstd9, lnv9, AF.Exp, scale=-0.5)
        negmr9 = sing.tile([128, 9], f32)
        nc.vector.tensor_mul(negmr9, mvall[:, :, 0], rstd9)
        nc.vector.tensor_scalar_mul(negmr9, negmr9, -1.0)
        for t_i in range(9):
            st_ = sts[t_i]
            slnt = sp.tile([128, CS], f32, tag="slnt")
            nc.scalar.activation(slnt, st_, AF.Identity,
                                 bias=negmr9[:, t_i:t_i + 1],
                                 scale=rstd9[:, t_i:t_i + 1])
            nc.vector.tensor_mul(slnt, slnt, gsb_t)
            slnb = sp.tile([128, CS], bf16, tag="slnb")
            nc.vector.tensor_add(slnb, slnt, bsb_t)
            for k in range(2):
                pt = pT.tile([128, 4, NB, BP], bf16, tag="ptb", name="pt")
                nc.tensor.transpose(pt[:, 0, :, :],
                                    slnb[:, k * 128:(k + 1) * 128], ident_bf)
                if t_i < 8:
                    dst = slnT[:, k, t_i * 128:(t_i + 1) * 128]
                else:
                    dst = slT[:, k, :]
                nc.scalar.activation(dst, pt[:, 0, :, :], AF.Copy)

        # ---------------- K^T (+cmask row), bf16 ----------------
        for m in range(2):
            for jh in range(2):
                ps = pP.tile([128, 4, 128], f32, tag="lp", name="ps")
                for k in range(2):
                    nc.tensor.matmul(
                        ps[:, :, :], wmats["wk"][:, k, m * 128:(m + 1) * 128],
                        slnT[:, k, jh * 512:(jh + 1) * 512],
                        start=(k == 0), stop=(k == 1))
                for q in range(4):
                    h = m * 4 + q
                    nc.scalar.activation(
                        kt[0:32, h, jh * 512:(jh + 1) * 512],
                        ps[q * 32:(q + 1) * 32, :, :], AF.Copy)
        for h in range(H):
            nc.sync.dma_start(out=kt[32:33, h, :], in_=cm_e[0:1, :])

        # ---------------- Q^T (+ones row), folded scale/bias ----------
        for m in range(2):
            ps = pP.tile([128, 4, 128], f32, tag="lp", name="ps")
            for k in range(2):
                nc.tensor.matmul(
                    ps[:, 0, :], wmats["wq"][:, k, m * 128:(m + 1) * 128],
                    slT[:, k, :], start=(k == 0), stop=(k == 1))
            for q in range(4):
                h = m * 4 + q
                nc.scalar.activation(
                    qt[0:32, h, :], ps[q * 32:(q + 1) * 32, 0, :], AF.Identity,
                    bias=bq_t[q * 32:(q + 1) * 32, m:m + 1])

        # ---------------- V (interleaved [V_h | 1]) ----------------
        for jc in range(8):
            ps = pP.tile([128, 4, 128], f32, tag="lp", name="ps")
            for k in range(2):
                nc.tensor.matmul(
                    ps[:, 0:2, :], slnT[:, k, jc * 128:(jc + 1) * 128],
                    wmats["wv"][:, k, :], start=(k == 0), stop=(k == 1))
            for h in range(H):
                nc.scalar.activation(
                    vsb[:, jc, h, 0:32],
                    ps[:, h // 4, (h % 4) * 32:(h % 4) * 32 + 32], AF.Copy)

        # ---------------- G = sigmoid(s_ln @ Wg) ----------------
        psg = pP.tile([128, 4, 128], f32, tag="lp", name="psg")
        for k in range(2):
            nc.tensor.matmul(psg[:, 0:2, :], slT[:, k, :], wmats["wg"][:, k, :],
                             start=(k == 0), stop=(k == 1))
        eg = sp.tile([128, CS], f32, tag="eg")
        nc.scalar.activation(eg, psg[:, 0:2, :], AF.Exp, scale=-1.0)
        nc.vector.tensor_scalar_add(eg, eg, 1.0)
        nc.vector.reciprocal(gate_g, eg)

        # ---------------- main z loop ----------------
        o_ps = pO.tile([128, H, 33], f32)

        for jc in range(8):
            j0 = jc * 128
            zt = zp.tile([CZ, LR, 128], bf16, tag="zt")  # [c, i, j]
            nc.sync.dma_start(out=zt, in_=zc_e[jc])

            b_sb = bbp.tile([128, 4, NB, 4, BH], bf16, tag="bsb")
            # [j, Q, b, ii, cc]; i = 32*Q + 4*b + ii  (i-ordered in memory)

            for half in range(2):
                i_h = 64 * half
                zq = sqp.tile([CZ, 64, 128], bf16, tag="zq")
                if (2 * jc + half) % 3 == 1:
                    nc.scalar.activation(zq, zt[:, i_h:i_h + 64, :], AF.Square)
                else:
                    nc.vector.tensor_mul(
                        zq, zt[:, i_h:i_h + 64, :], zt[:, i_h:i_h + 64, :])
                bkt = pB.tile([128, 2, 4, 128], f32, tag="bB", name="bkt")
                for b in range(NB):
                    for q in range(2):
                        i0 = i_h + 32 * q + 4 * b
                        nc.tensor.matmul(
                            bkt[:, q, :, :], stz_t[:, b, :],
                            zt[:, i0:i0 + 4, :],
                            start=(b == 0), stop=False, skip_group_check=True)
                for b in range(NB):
                    for q in range(2):
                        i0 = 32 * q + 4 * b
                        nc.tensor.matmul(
                            bkt[:, q, :, :], stq_t[:, b, :],
                            zq[:, i0:i0 + 4, :],
                            start=False, stop=(b == NB - 1),
                            skip_group_check=True)
                bs = bsp.tile([128, 2, 4, 128], bf16, tag="bs")
                nc.scalar.activation(bs[:, 0], bkt[:, 0], AF.Copy)
                nc.scalar.activation(bs[:, 1], bkt[:, 1], AF.Copy)
                for q in range(2):
                    ptb = pT.tile([128, 4, NB, BP], bf16, tag="ptb")
                    for u in range(4):
                        nc.tensor.transpose(
                            ptb[:, u, :, :], bs[:, q, u, :], ident_bf)
                    nc.vector.tensor_copy(
                        out=b_sb[:, 2 * half + q, :, :, :],
                        in_=ptb[:, :, :, 0:BH].rearrange(
                            "j i2 b c -> j b i2 c"))

            # ---- rstd^T: var = sumsq/128 - mean^2; rsqrt on DVE ----
            mean = b_sb[:, :, :, :, 8]
            ssq = b_sb[:, :, :, :, 9]
            m2 = wkp.tile([128, 128], f32, tag="m2")
            nc.vector.tensor_mul(m2, mean, mean)
            vr = wkp.tile([128, 128], f32, tag="vr")
            nc.vector.scalar_tensor_tensor(
                vr, ssq, 1.0 / CZ, m2, mybir.AluOpType.mult,
                mybir.AluOpType.subtract)
            sr = wkp.tile([128, 128], mybir.dt.int32, tag="sr")
            nc.vector.tensor_scalar(
                sr, vr.bitcast(mybir.dt.int32), 1, None,
                mybir.AluOpType.arith_shift_right)
            y0i = wkp.tile([128, 128], mybir.dt.int32, tag="y0i")
            nc.vector.tensor_scalar(
                y0i, sr, -1, 0x5F3759DF, mybir.AluOpType.mult,
                mybir.AluOpType.add)
            y0 = y0i.bitcast(f32)
            t1 = wkp.tile([128, 128], f32, tag="t1")
            nc.vector.tensor_mul(t1, y0, y0)
            nc.vector.tensor_mul(t1, t1, vr)
            nc.vector.tensor_scalar(
                t1, t1, -0.5, 1.5, mybir.AluOpType.mult, mybir.AluOpType.add)
            rstd = wkp.tile([128, 128], f32, tag="rstd")
            nc.vector.tensor_mul(rstd, y0, t1)

            # ---- attention for this chunk, transposed [j, i] ----
            for hq in range(2):
                lp = pP.tile([128, 4, 128], f32, tag="lp")
                for hh in range(4):
                    h = 4 * hq + hh
                    nc.tensor.matmul(lp[:, hh, :], kt[:, h, j0:j0 + 128],
                                     qt[:, h, :], start=True, stop=True)
                for hh in range(4):
                    h = 4 * hq + hh
                    rb = wkp.tile([128, 128], f32, tag="rb")
                    nc.gpsimd.tensor_mul(rb, b_sb[:, :, :, :, h], rstd)
                    lg = wkp.tile([128, 128], f32, tag="lg")
                    nc.vector.tensor_add(lg, rb, lp[:, hh, :])
                    ex = expp.tile([128, 128], bf16, tag="ex")
                    nc.scalar.activation(ex, lg, AF.Exp)
                    nc.tensor.matmul(
                        o_ps[:, h, :], ex, vsb[:, jc, h, :],
                        start=(jc == 0 and h == 0),
                        stop=(jc == 7 and h == 7), skip_group_check=True)

        # ---------------- epilogue ----------------
        den = sing.tile([128, H], f32)
        nc.scalar.activation(den, o_ps[:, :, 32], AF.Copy)
        rec = sing.tile([128, H], f32)
        nc.vector.reciprocal(rec, den)
        gated = sing.tile([128, CS], f32)
        for h in range(H):
            nc.vector.tensor_scalar_mul(
                gated[:, h * 32:(h + 1) * 32],
                o_ps[:, h, 0:32], rec[:, h:h + 1])
        nc.vector.tensor_mul(gated, gated, gate_g)
        gated_b = sing.tile([128, CS], bf16)
        nc.scalar.activation(gated_b, gated, AF.Copy)
        gts = []
        for k in range(2):
            pt = pT.tile([128, 4, NB, BP], bf16, tag="ptb", name="pt")
            nc.tensor.transpose(pt[:, 0, :, :],
                                gated_b[:, k * 128:(k + 1) * 128], ident_bf)
            gt = wkp.tile([128, 128], bf16, tag=f"gt{k}")
            nc.scalar.activation(gt, pt[:, 0, :, :], AF.Copy)
            gts.append(gt)
        dsps = pP.tile([128, 4, 128], f32, tag="lp", name="ps")
        for k in range(2):
            nc.tensor.matmul(dsps[:, 0:2, :], gts[k], wmats["wo"][:, k, :],
                             start=(k == 0), stop=(k == 1))
        dsb = sing.tile([128, CS], f32)
        nc.scalar.activation(dsb, dsps[:, 0:2, :], AF.Copy)
        nc.vector.tensor_scalar_mul(dsb, dsb, rm_t[:, 0:1])
        nc.sync.dma_start(out=out_e[:, :], in_=dsb)

    nc.compile()
    return nc


def _host_prep(s, z, res_mask, g_s, b_s, g_z, b_z, Wq, bq, Wk, Wv, Wb, Wg, Wo):
    import ml_dtypes
    bf16 = ml_dtypes.bfloat16

    s = np.ascontiguousarray(np.asarray(s, np.float32))
    res_mask = np.asarray(res_mask).astype(bool)
    g_s = np.asarray(g_s, np.float32)
    b_s = np.asarray(b_s, np.float32)
    g_z = np.asarray(g_z, np.float32)
    b_z = np.asarray(b_z, np.float32)
    scale = 1.0 / np.sqrt(D)
    wqp = (np.asarray(Wq, np.float32) * scale).astype(bf16)
    bqp = np.ascontiguousarray(
        (np.asarray(bq, np.float32) * scale).reshape(CS, 1))
    wb = np.asarray(Wb, np.float32)
    w1 = g_z[:, None] * wb
    u = g_z @ wb
    wb2 = (w1 - u[None, :] / CZ).astype(np.float32)  # [CZ, H]

    # band stationaries [c, b, 128] (16 cols per band, 10 meaningful)
    stz = np.zeros((CZ, NB, NB * BP), np.float32)
    stq = np.zeros((CZ, NB, NB * BP), np.float32)
    for b in range(NB):
        stz[:, b, BP * b:BP * b + H] = wb2
        stz[:, b, BP * b + 8] = 1.0 / CZ
        stq[:, b, BP * b + 9] = 1.0

    cmask = np.where(res_mask, 0.0, -1e30).astype(np.float32).reshape(1, L)
    rmask = res_mask.astype(np.float32)
    gsb = np.ascontiguousarray(np.broadcast_to(g_s, (128, CS)))
    bsb = np.ascontiguousarray(np.broadcast_to(b_s, (128, CS)))

    common = {
        "s": s,
        "wq": np.ascontiguousarray(wqp),
        "bq": bqp,
        "wk": np.ascontiguousarray(np.asarray(Wk, np.float32).astype(bf16)),
        "wv": np.ascontiguousarray(np.asarray(Wv, np.float32).astype(bf16)),
        "wg": np.ascontiguousarray(np.asarray(Wg, np.float32).astype(bf16)),
        "wo": np.ascontiguousarray(np.asarray(Wo, np.float32).astype(bf16)),
        "stz": np.ascontiguousarray(stz.astype(bf16)),
        "stq": np.ascontiguousarray(stq.astype(bf16)),
        "cm": np.ascontiguousarray(cmask.astype(bf16)),
        "gsb": gsb, "bsb": bsb,
    }

    zb = np.asarray(z, np.float32).astype(bf16)  # [i, j, c] bf16
    in_maps = []
    for r in range(NCORES):
        zc_r = np.ascontiguousarray(
            zb[r * LR:(r + 1) * LR].reshape(LR, 8, 128, CZ)
            .transpose(1, 3, 0, 2))  # [jc, c, i_local, j]
        m = dict(common)
        m["zc"] = zc_r
        m["sl"] = np.ascontiguousarray(s[r * LR:(r + 1) * LR])
        m["rm"] = np.ascontiguousarray(
            rmask[r * LR:(r + 1) * LR].reshape(LR, 1))
        in_maps.append(m)
    return in_maps


def kernel(s, z, res_mask, g_s, b_s, g_z, b_z, Wq, bq, Wk, Wv, Wb, Wg, Wo):
    global LAST_RESULT
    in_maps = _host_prep(s, z, res_mask, g_s, b_s, g_z, b_z,
                         Wq, bq, Wk, Wv, Wb, Wg, Wo)

    if "nc" not in _CACHE:
        _CACHE["nc"] = _build_graph()
    nc = _CACHE["nc"]

    from concourse.bass_utils import run_bass_kernel_spmd

    import os
    tmpdir = os.environ.get("BASS_TMPDIR")
    if tmpdir:
        os.makedirs(tmpdir, exist_ok=True)
    res = run_bass_kernel_spmd(nc, in_maps, core_ids=list(range(NCORES)),
                               tmpdir=tmpdir)
    LAST_RESULT = res
    out = np.concatenate([np.asarray(res.results[r]["out"])
                          for r in range(NCORES)], axis=0)
    return out.astype(np.float32)


# revision 12
# speedup vs baseline: 2.8559x; 1.0153x over previous
"""AttentionPairBias distributed Trainium2 kernel (8 NeuronCores), v2.

Sequence-parallel over query rows: core r owns i-rows [r*128, (r+1)*128).
z is host-cast to bf16 and reordered to [c, j, i_local] so each DMA is a
fully-contiguous 32KB/partition run and z arrives with c on partitions.

Pair-bias path (the expensive part) is computed with z as the MOVING
matmul operand against a small zero-padded stationary:
  stz[c, b, 10b+h] = wb2'[c,h] (h<8), 1/128 at col 10b+8   (mean)
  stq[c, b, 10b+9] = 1                                      (sum of z^2)
Eight band-matmuls (+ eight for z^2) accumulate into one PSUM bank
giving Bt[(b,h), (j,g)] for i = 8g+b -- B transposed, plus per-(i,j)
mean and sumsq rows.  Bands are PE-transposed in groups of 4 to get
[j, (g,b,h)] tiles.  Attention then runs in transposed [j, i] layout
(QK operands swapped); exp(logits) feeds attn@[V|1] directly as the
stationary, so no per-tile attention transposes and the softmax
denominator falls out of the ones column.

Math folding (host):
  z_ln @ Wb = rstd * (z @ Wb2) + v_h  with Wb2 = g_z*Wb - (g_z@Wb)/128,
  v_h = b_z@Wb constant per h cancels in softmax.  Q scale 1/sqrt(D)
  folded into Wq/bq.  Column mask folded into kt row 32 x qt ones row.
"""

import sys

if "/opt/trn_rl_repo" not in sys.path:
    sys.path.insert(0, "/opt/trn_rl_repo")

import numpy as np

L = 1024
CS = 256
CZ = 128
H = 8
D = 32
NCORES = 8
LR = L // NCORES  # 128
EPS = 1e-5
NB = 8            # i-bands per band-matmul group (i = 8g + b)
BH = 10           # meaningful cols per band: 8 heads + mean + sumsq
BP = 16           # padded cols per band (NB*BP = 128 -> FWL-eligible)
NGT = 16          # g values per j-chunk (128 i / NB)

_CACHE = {}
LAST_RESULT = None


def _build_graph():
    from contextlib import ExitStack

    import concourse.mybir as mybir
    import concourse.tile as tile
    from concourse import bacc
    from concourse.masks import make_identity

    f32 = mybir.dt.float32
    bf16 = mybir.dt.bfloat16
    AF = mybir.ActivationFunctionType

    nc = bacc.Bacc("TRN2", target_bir_lowering=False, debug=False)

    zc_e = nc.declare_dram_parameter("zc", [8, CZ, LR, 128], bf16,
                                     isOutput=False)
    s_e = nc.declare_dram_parameter("s", [L, CS], f32, isOutput=False)
    sl_e = nc.declare_dram_parameter("sl", [LR, CS], f32, isOutput=False)
    wq_e = nc.declare_dram_parameter("wq", [CS, CS], bf16, isOutput=False)
    bq_e = nc.declare_dram_parameter("bq", [CS, 1], f32, isOutput=False)
    wk_e = nc.declare_dram_parameter("wk", [CS, CS], bf16, isOutput=False)
    wv_e = nc.declare_dram_parameter("wv", [CS, CS], bf16, isOutput=False)
    wg_e = nc.declare_dram_parameter("wg", [CS, CS], bf16, isOutput=False)
    wo_e = nc.declare_dram_parameter("wo", [CS, CS], bf16, isOutput=False)
    stz_e = nc.declare_dram_parameter("stz", [CZ, NB, NB * BP], bf16,
                                      isOutput=False)
    stq_e = nc.declare_dram_parameter("stq", [CZ, NB, NB * BP], bf16,
                                      isOutput=False)
    cm_e = nc.declare_dram_parameter("cm", [1, L], bf16, isOutput=False)
    rm_e = nc.declare_dram_parameter("rm", [LR, 1], f32, isOutput=False)
    gsb_e = nc.declare_dram_parameter("gsb", [128, CS], f32, isOutput=False)
    bsb_e = nc.declare_dram_parameter("bsb", [128, CS], f32, isOutput=False)
    out_e = nc.declare_dram_parameter("out", [LR, CS], f32, isOutput=True)

    with ExitStack() as ctx:
        tc = ctx.enter_context(tile.TileContext(nc))
        sing = ctx.enter_context(tc.tile_pool(name="sing", bufs=1))
        zp = ctx.enter_context(tc.tile_pool(name="zp", bufs=2))
        sqp = ctx.enter_context(tc.tile_pool(name="sqp", bufs=2))
        bsp = ctx.enter_context(tc.tile_pool(name="bsp", bufs=2))
        bbp = ctx.enter_context(tc.tile_pool(name="bbp", bufs=2))
        wkp = ctx.enter_context(tc.tile_pool(name="wkp", bufs=4))
        expp = ctx.enter_context(tc.tile_pool(name="expp", bufs=3))
        sp = ctx.enter_context(tc.tile_pool(name="sp", bufs=3))
        pB = ctx.enter_context(tc.tile_pool(name="pB", bufs=2, space="PSUM"))
        pT = ctx.enter_context(tc.tile_pool(name="pT", bufs=2, space="PSUM"))
        pP = ctx.enter_context(tc.tile_pool(name="pP", bufs=1, space="PSUM"))
        pO = ctx.enter_context(tc.tile_pool(name="pO", bufs=1, space="PSUM"))

        # ---------------- constants / params ----------------
        for cval in (0.0, 1.0, -1.0, -0.5, EPS):
            cl = sing.tile([128, 1], f32, tag=f"const_{cval}")
            nc.vector.memset(cl, cval)
            nc.const_aps.aps[(f32, cval)] = cl[:, :]

        ident = sing.tile([128, 128], f32)
        make_identity(nc, ident)

        stz_t = sing.tile([CZ, NB, NB * BP], bf16, tag="stz")
        nc.sync.dma_start(out=stz_t, in_=stz_e[:, :, :])
        stq_t = sing.tile([CZ, NB, NB * BP], bf16, tag="stq")
        nc.sync.dma_start(out=stq_t, in_=stq_e[:, :, :])
        ident_bf = sing.tile([128, 128], bf16)
        nc.scalar.activation(ident_bf, ident, AF.Copy)
        gsb_t = sing.tile([128, CS], f32)
        nc.sync.dma_start(out=gsb_t, in_=gsb_e[:, :])
        bsb_t = sing.tile([128, CS], f32)
        nc.sync.dma_start(out=bsb_t, in_=bsb_e[:, :])
        rm_t = sing.tile([LR, 1], f32)
        nc.sync.dma_start(out=rm_t, in_=rm_e[:, :])
        bq_t = sing.tile([128, 2], f32)
        nc.sync.dma_start(out=bq_t, in_=bq_e.rearrange("(k p) x -> p (k x)", p=128))

        wmats = {}
        for name, e, dt_ in (("wq", wq_e, bf16), ("wk", wk_e, bf16),
                             ("wv", wv_e, bf16), ("wg", wg_e, bf16),
                             ("wo", wo_e, bf16)):
            t = sing.tile([128, 2, CS], dt_, tag=f"w_{name}")
            nc.sync.dma_start(out=t, in_=e.rearrange("(k p) n -> p k n", p=128))
            wmats[name] = t

        # ---------------- persistent SBUF state ----------------
        slnT = sing.tile([128, 2, L], bf16)       # s_ln^T  [cs_in, j]
        slT = sing.tile([128, 2, LR], bf16)       # s_ln^T of local rows
        kt = sing.tile([33, H, L], bf16)          # [K_h^T ; cmask]
        qt = sing.tile([33, H, LR], bf16)         # [Q_h^T ; ones]
        vsb = sing.tile([128, NCORES, H, 33], bf16)  # per j-chunk [V_h | 1]
        gate_g = sing.tile([LR, CS], f32)         # sigmoid gate

        nc.vector.memset(vsb, 1.0)  # ones columns (:, :, :, 32) survive
        nc.vector.memset(qt[32:33, :, :], 1.0)

        # ---------------- s-side layernorm + transpose ----------------
        sts = []
        mvall = sing.tile([128, 9, 2], f32)
        for t_i in range(9):
            src = s_e[t_i * 128:(t_i + 1) * 128, :] if t_i < 8 else sl_e[:, :]
            st_ = sp.tile([128, CS], f32, tag=f"st{t_i}")
            nc.sync.dma_start(out=st_, in_=src)
            sts.append(st_)
            st6 = wkp.tile([128, 6], f32, tag="st6")
            nc.vector.bn_stats(out=st6, in_=st_)
            nc.vector.bn_aggr(out=mvall[:, t_i, :], in_=st6)
        lnv9 = sing.tile([128, 9], f32)
        nc.scalar.activation(lnv9, mvall[:, :, 1], AF.Ln, bias=EPS)
        rstd9 = sing.tile([128, 9], f32)
        nc.scalar.activation(rstd9, lnv9, AF.Exp, scale=-0.5)
        negmr9 = sing.tile([128, 9], f32)
        nc.vector.tensor_mul(negmr9, mvall[:, :, 0], rstd9)
        nc.vector.tensor_scalar_mul(negmr9, negmr9, -1.0)
        for t_i in range(9):
            st_ = sts[t_i]
            slnt = sp.tile([128, CS], f32, tag="slnt")
            nc.scalar.activation(slnt, st_, AF.Identity,
                                 bias=negmr9[:, t_i:t_i + 1],
                                 scale=rstd9[:, t_i:t_i + 1])
            nc.vector.tensor_mul(slnt, slnt, gsb_t)
            slnb = sp.tile([128, CS], bf16, tag="slnb")
            nc.vector.tensor_add(slnb, slnt, bsb_t)
            for k in range(2):
                pt = pT.tile([128, 4, NB, BP], bf16, tag="ptb", name="pt")
                nc.tensor.transpose(pt[:, 0, :, :],
                                    slnb[:, k * 128:(k + 1) * 128], ident_bf)
                if t_i < 8:
                    dst = slnT[:, k, t_i * 128:(t_i + 1) * 128]
                else:
                    dst = slT[:, k, :]
                nc.scalar.activation(dst, pt[:, 0, :, :], AF.Copy)

        # ---------------- K^T (+cmask row), bf16 ----------------
        for m in range(2):
            for jh in range(2):
                ps = pP.tile([128, 4, 128], f32, tag="lp", name="ps")
                for k in range(2):
                    nc.tensor.matmul(
                        ps[:, :, :], wmats["wk"][:, k, m * 128:(m + 1) * 128],
                        slnT[:, k, jh * 512:(jh + 1) * 512],
                        start=(k == 0), stop=(k == 1))
                for q in range(4):
                    h = m * 4 + q
                    nc.scalar.activation(
                        kt[0:32, h, jh * 512:(jh + 1) * 512],
                        ps[q * 32:(q + 1) * 32, :, :], AF.Copy)
        for h in range(H):
            nc.sync.dma_start(out=kt[32:33, h, :], in_=cm_e[0:1, :])

        # ---------------- Q^T (+ones row), folded scale/bias ----------
        for m in range(2):
            ps = pP.tile([128, 4, 128], f32, tag="lp", name="ps")
            for k in range(2):
                nc.tensor.matmul(
                    ps[:, 0, :], wmats["wq"][:, k, m * 128:(m + 1) * 128],
                    slT[:, k, :], start=(k == 0), stop=(k == 1))
            for q in range(4):
                h = m * 4 + q
                nc.scalar.activation(
                    qt[0:32, h, :], ps[q * 32:(q + 1) * 32, 0, :], AF.Identity,
                    bias=bq_t[q * 32:(q + 1) * 32, m:m + 1])

        # ---------------- V (interleaved [V_h | 1]) ----------------
        for jc in range(8):
            ps = pP.tile([128, 4, 128], f32, tag="lp", name="ps")
            for k in range(2):
                nc.tensor.matmul(
                    ps[:, 0:2, :], slnT[:, k, jc * 128:(jc + 1) * 128],
                    wmats["wv"][:, k, :], start=(k == 0), stop=(k == 1))
            for h in range(H):
                nc.scalar.activation(
                    vsb[:, jc, h, 0:32],
                    ps[:, h // 4, (h % 4) * 32:(h % 4) * 32 + 32], AF.Copy)

        # ---------------- G = sigmoid(s_ln @ Wg) ----------------
        psg = pP.tile([128, 4, 128], f32, tag="lp", name="psg")
        for k in range(2):
            nc.tensor.matmul(psg[:, 0:2, :], slT[:, k, :], wmats["wg"][:, k, :],
                             start=(k == 0), stop=(k == 1))
        eg = sp.tile([128, CS], f32, tag="eg")
        nc.scalar.activation(eg, psg[:, 0:2, :], AF.Exp, scale=-1.0)
        nc.vector.tensor_scalar_add(eg, eg, 1.0)
        nc.vector.reciprocal(gate_g, eg)

        # ---------------- main z loop ----------------
        o_ps = pO.tile([128, H, 33], f32)

        for jc in range(8):
            j0 = jc * 128
            zt = zp.tile([CZ, LR, 128], bf16, tag="zt")  # [c, i, j]
            nc.sync.dma_start(out=zt, in_=zc_e[jc])

            b_sb = bbp.tile([128, 4, NB, 4, BH], bf16, tag="bsb")
            # [j, Q, b, ii, cc]; i = 32*Q + 4*b + ii  (i-ordered in memory)

            for half in range(2):
                i_h = 64 * half
                zq = sqp.tile([CZ, 64, 128], bf16, tag="zq")
                nc.vector.tensor_mul(
                    zq, zt[:, i_h:i_h + 64, :], zt[:, i_h:i_h + 64, :])
                bkt = pB.tile([128, 2, 4, 128], f32, tag="bB", name="bkt")
                for b in range(NB):
                    for q in range(2):
                        i0 = i_h + 32 * q + 4 * b
                        nc.tensor.matmul(
                            bkt[:, q, :, :], stz_t[:, b, :],
                            zt[:, i0:i0 + 4, :],
                            start=(b == 0), stop=False, skip_group_check=True)
                for b in range(NB):
                    for q in range(2):
                        i0 = 32 * q + 4 * b
                        nc.tensor.matmul(
                            bkt[:, q, :, :], stq_t[:, b, :],
                            zq[:, i0:i0 + 4, :],
                            start=False, stop=(b == NB - 1),
                            skip_group_check=True)
                bs = bsp.tile([128, 2, 4, 128], bf16, tag="bs")
                nc.scalar.activation(bs[:, 0], bkt[:, 0], AF.Copy)
                nc.scalar.activation(bs[:, 1], bkt[:, 1], AF.Copy)
                for q in range(2):
                    ptb = pT.tile([128, 4, NB, BP], bf16, tag="ptb")
                    for u in range(4):
                        nc.tensor.transpose(
                            ptb[:, u, :, :], bs[:, q, u, :], ident_bf)
                    nc.vector.tensor_copy(
                        out=b_sb[:, 2 * half + q, :, :, :],
                        in_=ptb[:, :, :, 0:BH].rearrange(
                            "j i2 b c -> j b i2 c"))

            # ---- rstd^T: var = sumsq/128 - mean^2; rsqrt on DVE ----
            mean = b_sb[:, :, :, :, 8]
            ssq = b_sb[:, :, :, :, 9]
            m2 = wkp.tile([128, 128], f32, tag="m2")
            nc.vector.tensor_mul(m2, mean, mean)
            vr = wkp.tile([128, 128], f32, tag="vr")
            nc.vector.scalar_tensor_tensor(
                vr, ssq, 1.0 / CZ, m2, mybir.AluOpType.mult,
                mybir.AluOpType.subtract)
            sr = wkp.tile([128, 128], mybir.dt.int32, tag="sr")
            nc.vector.tensor_scalar(
                sr, vr.bitcast(mybir.dt.int32), 1, None,
                mybir.AluOpType.arith_shift_right)
            y0i = wkp.tile([128, 128], mybir.dt.int32, tag="y0i")
            nc.vector.tensor_scalar(
                y0i, sr, -1, 0x5F3759DF, mybir.AluOpType.mult,
                mybir.AluOpType.add)
            y0 = y0i.bitcast(f32)
            t1 = wkp.tile([128, 128], f32, tag="t1")
            nc.vector.tensor_mul(t1, y0, y0)
            nc.vector.tensor_mul(t1, t1, vr)
            nc.vector.tensor_scalar(
                t1, t1, -0.5, 1.5, mybir.AluOpType.mult, mybir.AluOpType.add)
            rstd = wkp.tile([128, 128], f32, tag="rstd")
            nc.vector.tensor_mul(rstd, y0, t1)

            # ---- attention for this chunk, transposed [j, i] ----
            for hq in range(2):
                lp = pP.tile([128, 4, 128], f32, tag="lp")
                for hh in range(4):
                    h = 4 * hq + hh
                    nc.tensor.matmul(lp[:, hh, :], kt[:, h, j0:j0 + 128],
                                     qt[:, h, :], start=True, stop=True)
                for hh in range(4):
                    h = 4 * hq + hh
                    rb = wkp.tile([128, 128], f32, tag="rb")
                    nc.gpsimd.tensor_mul(rb, b_sb[:, :, :, :, h], rstd)
                    lg = wkp.tile([128, 128], f32, tag="lg")
                    nc.vector.tensor_add(lg, rb, lp[:, hh, :])
                    ex = expp.tile([128, 128], bf16, tag="ex")
                    nc.scalar.activation(ex, lg, AF.Exp)
                    nc.tensor.matmul(
                        o_ps[:, h, :], ex, vsb[:, jc, h, :],
                        start=(jc == 0 and h == 0),
                        stop=(jc == 7 and h == 7), skip_group_check=True)

        # ---------------- epilogue ----------------
        den = sing.tile([128, H], f32)
        nc.scalar.activation(den, o_ps[:, :, 32], AF.Copy)
        rec = sing.tile([128, H], f32)
        nc.vector.reciprocal(rec, den)
        gated = sing.tile([128, CS], f32)
        for h in range(H):
            nc.vector.tensor_scalar_mul(
                gated[:, h * 32:(h + 1) * 32],
                o_ps[:, h, 0:32], rec[:, h:h + 1])
        nc.vector.tensor_mul(gated, gated, gate_g)
        gated_b = sing.tile([128, CS], bf16)
        nc.scalar.activation(gated_b, gated, AF.Copy)
        gts = []
        for k in range(2):
            pt = pT.tile([128, 4, NB, BP], bf16, tag="ptb", name="pt")
            nc.tensor.transpose(pt[:, 0, :, :],
                                gated_b[:, k * 128:(k + 1) * 128], ident_bf)
            gt = wkp.tile([128, 128], bf16, tag=f"gt{k}")
            nc.scalar.activation(gt, pt[:, 0, :, :], AF.Copy)
            gts.append(gt)
        dsps = pP.tile([128, 4, 128], f32, tag="lp", name="ps")
        for k in range(2):
            nc.tensor.matmul(dsps[:, 0:2, :], gts[k], wmats["wo"][:, k, :],
                             start=(k == 0), stop=(k == 1))
        dsb = sing.tile([128, CS], f32)
        nc.scalar.activation(dsb, dsps[:, 0:2, :], AF.Copy)
        nc.vector.tensor_scalar_mul(dsb, dsb, rm_t[:, 0:1])
        nc.sync.dma_start(out=out_e[:, :], in_=dsb)

    nc.compile()
    return nc


def _host_prep(s, z, res_mask, g_s, b_s, g_z, b_z, Wq, bq, Wk, Wv, Wb, Wg, Wo):
    import ml_dtypes
    bf16 = ml_dtypes.bfloat16

    s = np.ascontiguousarray(np.asarray(s, np.float32))
    res_mask = np.asarray(res_mask).astype(bool)
    g_s = np.asarray(g_s, np.float32)
    b_s = np.asarray(b_s, np.float32)
    g_z = np.asarray(g_z, np.float32)
    b_z = np.asarray(b_z, np.float32)
    scale = 1.0 / np.sqrt(D)
    wqp = (np.asarray(Wq, np.float32) * scale).astype(bf16)
    bqp = np.ascontiguousarray(
        (np.asarray(bq, np.float32) * scale).reshape(CS, 1))
    wb = np.asarray(Wb, np.float32)
    w1 = g_z[:, None] * wb
    u = g_z @ wb
    wb2 = (w1 - u[None, :] / CZ).astype(np.float32)  # [CZ, H]

    # band stationaries [c, b, 128] (16 cols per band, 10 meaningful)
    stz = np.zeros((CZ, NB, NB * BP), np.float32)
    stq = np.zeros((CZ, NB, NB * BP), np.float32)
    for b in range(NB):
        stz[:, b, BP * b:BP * b + H] = wb2
        stz[:, b, BP * b + 8] = 1.0 / CZ
        stq[:, b, BP * b + 9] = 1.0

    cmask = np.where(res_mask, 0.0, -1e30).astype(np.float32).reshape(1, L)
    rmask = res_mask.astype(np.float32)
    gsb = np.ascontiguousarray(np.broadcast_to(g_s, (128, CS)))
    bsb = np.ascontiguousarray(np.broadcast_to(b_s, (128, CS)))

    common = {
        "s": s,
        "wq": np.ascontiguousarray(wqp),
        "bq": bqp,
        "wk": np.ascontiguousarray(np.asarray(Wk, np.float32).astype(bf16)),
        "wv": np.ascontiguousarray(np.asarray(Wv, np.float32).astype(bf16)),
        "wg": np.ascontiguousarray(np.asarray(Wg, np.float32).astype(bf16)),
        "wo": np.ascontiguousarray(np.asarray(Wo, np.float32).astype(bf16)),
        "stz": np.ascontiguousarray(stz.astype(bf16)),
        "stq": np.ascontiguousarray(stq.astype(bf16)),
        "cm": np.ascontiguousarray(cmask.astype(bf16)),
        "gsb": gsb, "bsb": bsb,
    }

    zb = np.asarray(z, np.float32).astype(bf16)  # [i, j, c] bf16
    in_maps = []
    for r in range(NCORES):
        zc_r = np.ascontiguousarray(
            zb[r * LR:(r + 1) * LR].reshape(LR, 8, 128, CZ)
            .transpose(1, 3, 0, 2))  # [jc, c, i_local, j]
        m = dict(common)
        m["zc"] = zc_r
        m["sl"] = np.ascontiguousarray(s[r * LR:(r + 1) * LR])
        m["rm"] = np.ascontiguousarray(
            rmask[r * LR:(r + 1) * LR].reshape(LR, 1))
        in_maps.append(m)
    return in_maps


def kernel(s, z, res_mask, g_s, b_s, g_z, b_z, Wq, bq, Wk, Wv, Wb, Wg, Wo):
    global LAST_RESULT
    in_maps = _host_prep(s, z, res_mask, g_s, b_s, g_z, b_z,
                         Wq, bq, Wk, Wv, Wb, Wg, Wo)

    if "nc" not in _CACHE:
        _CACHE["nc"] = _build_graph()
    nc = _CACHE["nc"]

    from concourse.bass_utils import run_bass_kernel_spmd

    import os
    tmpdir = os.environ.get("BASS_TMPDIR")
    if tmpdir:
        os.makedirs(tmpdir, exist_ok=True)
    res = run_bass_kernel_spmd(nc, in_maps, core_ids=list(range(NCORES)),
                               tmpdir=tmpdir)
    LAST_RESULT = res
    out = np.concatenate([np.asarray(res.results[r]["out"])
                          for r in range(NCORES)], axis=0)
    return out.astype(np.float32)
